# revision 19
# baseline (speedup 1.0000x reference)
"""GCN pipeline (proj + 2x GCNConv + GraphNorm + spot-softmax aggregation +
MLP head) on 8 trn2 NeuronCores via Bass/Tile.

Nodes are relabeled spot-sorted on the host so core c owns spots
[c*SSH,(c+1)*SSH) and exactly the nodes belonging to those spots (padded to
NSHp). GCN tables are bf16 [*, 128] rows (256B), AllGathered, then dst-sorted
edges are gathered by src (dma_gather) and scatter-accumulated per dst tile
with feature-major one-hot matmuls (psum [H, 128dst] adds straight into agg).
The spot softmax-aggregation is core-local: static host-built one-hot
matrices turn it into ~50 small matmuls with no gathers.
"""
import sys, os
sys.path.insert(0, '/opt/trn_rl_repo')
import numpy as np

N_CORES = 8
HALF_BITS = 15


class Cfg:
    def __init__(self, n_nodes=50000, n_edges=800000, in_dim=128, hid=96,
                 attn_hid=32, out_dim=16, n_spots=5000, eps=1e-5):
        assert n_spots % N_CORES == 0
        self.N, self.E, self.IN, self.H = n_nodes, n_edges, in_dim, hid
        self.AH, self.OD, self.S, self.EPS = attn_hid, out_dim, n_spots, eps
        self.SSH = n_spots // N_CORES
        self.ST = (self.SSH + 127) // 128           # spot groups per core
        self.TROWB = 128                            # table row bf16 (256B)
        self.CALL = 4096                            # gather slots per call
        # filled after relabel:
        self.NSHp = None
        self.NT = None
        self.HALF = None


def _relabel(cfg, cts):
    """Spot-sort nodes; returns perm (new->old), per-core start/count, NSHp."""
    perm = np.argsort(cts, kind='stable')
    spot_sorted = cts[perm]
    # core boundaries: first node whose spot >= c*SSH
    starts = np.searchsorted(spot_sorted, np.arange(N_CORES + 1) * cfg.SSH)
    counts = np.diff(starts)
    NSHp = int(((counts.max() + 127) // 128) * 128)
    cfg.NSHp = NSHp
    cfg.NT = NSHp // 128
    cfg.HALF = (N_CORES // 2) * NSHp
    assert cfg.HALF < (1 << HALF_BITS), cfg.HALF
    return perm, starts[:-1], counts, spot_sorted


def _wrap_idx(flat):
    n = len(flat)
    assert n % 16 == 0
    w = flat.reshape(n // 16, 16).T.astype(np.int16)
    return np.tile(w, (8, 1))


def _prep_edges(cfg, src_gid, dst_new, node_core, starts):
    """Per-core dst-sorted edge structure, uniform across cores for SPMD.

    src_gid: per-edge global table row id (core*NSHp + local).
    dst_new: per-edge new (relabeled) dst node id.
    Block order per core: phase A (src_gid < HALF): tiles 0..NT-1, S[t,0]
    blocks each; then phase B likewise with S[t,1].
    """
    NSHp, NT, HALF = cfg.NSHp, cfg.NT, cfg.HALF
    core_of = node_core[dst_new]
    per_core = []
    cnt_all = np.zeros((N_CORES, NT, 2), np.int64)
    for c in range(N_CORES):
        m = core_of == c
        s_c = src_gid[m]
        d_c = dst_new[m] - starts[c]
        t_c = d_c // 128
        h_c = (s_c >= HALF).astype(np.int64)
        key = t_c * 2 + h_c
        order = np.argsort(key, kind='stable')
        per_core.append((s_c[order], d_c[order], key[order]))
        cnt_all[c] = np.bincount(key, minlength=NT * 2).reshape(NT, 2)
    S = (cnt_all.max(axis=0) + 127) // 128
    nblk = [int(S[:, h].sum()) for h in (0, 1)]
    slots = [n * 128 for n in nblk]
    b0 = np.zeros((NT, 2), np.int64)
    for h in (0, 1):
        b0[:, h] = np.cumsum(S[:, h]) - S[:, h]
    idx_w, dl_w = [], []
    for c in range(N_CORES):
        s_c, d_c, key = per_core[c]
        bounds = np.searchsorted(key, np.arange(NT * 2 + 1))
        idx_flat = np.zeros(slots[0] + slots[1], np.int64)
        dl_flat = -np.ones(slots[0] + slots[1], np.float32)
        for t in range(NT):
            for h in (0, 1):
                lo, hi = bounds[t * 2 + h], bounds[t * 2 + h + 1]
                n = hi - lo
                off = (0 if h == 0 else slots[0]) + int(b0[t, h]) * 128
                ss = s_c[lo:hi] - (0 if h == 0 else HALF)
                idx_flat[off:off + n] = ss
                dl_flat[off:off + n] = (d_c[lo:hi] % 128).astype(np.float32)
        idx_w.append(_wrap_idx(idx_flat))
        dl_w.append(np.ascontiguousarray(
            dl_flat.reshape(-1, 128).T))            # [128, nblk_tot]
    meta = dict(S=S, b0=b0, nblk=nblk, slots=slots)
    return idx_w, dl_w, meta


def _prep_spot_oh(cfg, spot_sorted, starts, counts):
    """Static one-hot matrices for the core-local spot aggregation.

    Returns the uniform (g, sg) pair list and per-core OH / OHT stacks:
    OH[k][128 node, 128 spot] for pair k=(g,sg); OHT[k] its transpose.
    """
    NT, ST, SSH = cfg.NT, cfg.ST, cfg.SSH
    pairs = set()
    percore_sp = []
    for c in range(N_CORES):
        sp_loc = spot_sorted[starts[c]:starts[c] + counts[c]] - c * SSH
        percore_sp.append(sp_loc)
        for g in range(NT):
            lo, hi = g * 128, min((g + 1) * 128, counts[c])
            if lo >= hi:
                continue
            for sg in range(sp_loc[lo] // 128, sp_loc[hi - 1] // 128 + 1):
                pairs.add((g, int(sg)))
    pairs = sorted(pairs)
    K = len(pairs)
    oh_l, oht_l = [], []
    for c in range(N_CORES):
        sp_loc = percore_sp[c]
        oh = np.zeros((128, K, 128), np.float32)
        for k, (g, sg) in enumerate(pairs):
            lo, hi = g * 128, min((g + 1) * 128, counts[c])
            for p in range(lo, hi):
                s = sp_loc[p - lo + lo]  # == sp_loc[p]
                if sg * 128 <= s < (sg + 1) * 128:
                    oh[p - lo, k, s - sg * 128] = 1.0
        oht = np.ascontiguousarray(oh.transpose(2, 1, 0))
        oh_l.append(oh.reshape(128, K * 128))
        oht_l.append(oht.reshape(128, K * 128))
    return pairs, oh_l, oht_l


def _calls(total_slots, call):
    out = []
    o = 0
    while o < total_slots:
        n = min(call, total_slots - o)
        out.append((o, n))
        o += n
    return out


def build_program(cfg, emeta, spairs):
    from concourse import bacc, mybir, tile

    f32, i16, bf16 = mybir.dt.float32, mybir.dt.int16, mybir.dt.bfloat16
    H, AH, OD = cfg.H, cfg.AH, cfg.OD
    NSHp, NT, SSH, ST = cfg.NSHp, cfg.NT, cfg.SSH, cfg.ST
    TROWB = cfg.TROWB
    S, nblk, slots = emeta['S'], emeta['nblk'], emeta['slots']
    K = len(spairs)

    nc = bacc.Bacc("TRN2", target_bir_lowering=False, debug=False,
                   num_devices=N_CORES, num_swdge_queues=4)

    def din(name, shape, dt=f32):
        return nc.dram_tensor(name, shape, dt, kind="ExternalInput")

    xT = din("xT", [cfg.IN, NSHp])
    deg_pp = din("deg_pp", [128, NT])
    deg_bc = din("deg_bc", [H, NSHp])
    iota16_in = din("iota16_in", [128, 128], bf16)
    ident_in = din("ident_in", [128, 128])
    idx_gcn = din("idx_gcn", [128, (slots[0] + slots[1]) // 16], i16)
    dl_gcn = din("dl_gcn", [128, nblk[0] + nblk[1]], bf16)
    oh_sp_in = din("oh_sp_in", [128, K * 128], bf16)
    oht_sp_in = din("oht_sp_in", [128, K * 128], bf16)
    projW = din("projW", [cfg.IN, H])
    W1, W2 = din("W1", [H, H]), din("W2", [H, H])
    attnW1, attnW2 = din("attnW1", [H, AH]), din("attnW2", [AH, 1])
    mlpW1, mlpW2 = din("mlpW1", [H, H]), din("mlpW2", [H, OD])
    pf = din("pf", [H, 16])
    attn_b1 = din("attn_b1", [AH, 1])
    attn_b2 = din("attn_b2", [1, 1])
    mlp_b2 = din("mlp_b2", [OD, 1])
    out = nc.dram_tensor("out", [SSH, OD], f32, kind="ExternalOutput")
    DEBUG = os.environ.get('KERNEL_DEBUG', '0') == '1'
    if DEBUG:
        dbg_h0 = nc.dram_tensor("dbg_h0", [H, NSHp], f32, kind="ExternalOutput")
        dbg_h1 = nc.dram_tensor("dbg_h1", [H, NSHp], f32, kind="ExternalOutput")
        dbg_h2 = nc.dram_tensor("dbg_h2", [H, NSHp], f32, kind="ExternalOutput")
        dbg_sc = nc.dram_tensor("dbg_sc", [1, NSHp], f32, kind="ExternalOutput")
        dbg_sp = nc.dram_tensor("dbg_sp", [H, ST * 128], f32,
                                kind="ExternalOutput")
        dbg_den = nc.dram_tensor("dbg_den", [128, ST], f32,
                                 kind="ExternalOutput")
        dbg_ag = [nc.dram_tensor(f"dbg_ag{l}", [H, NSHp], f32,
                                 kind="ExternalOutput") for l in range(2)]

    gcalls = [_calls(slots[0], cfg.CALL), _calls(slots[1], cfg.CALL)]
    CB = cfg.CALL // 128                     # blocks per full call

    with tile.TileContext(nc) as tc:
        with (
            tc.tile_pool(name="res", bufs=1) as res,
            tc.tile_pool(name="gat", bufs=3) as gat,
            tc.tile_pool(name="ohp", bufs=2) as ohp,
            tc.tile_pool(name="stg", bufs=3) as stg,
            tc.tile_pool(name="spp", bufs=2) as spp,
            tc.tile_pool(name="mmp", bufs=2, space="PSUM") as mmp,
            tc.tile_pool(name="scp", bufs=2, space="PSUM") as scp,
            tc.tile_pool(name="dram", bufs=1, space="DRAM") as dram,
        ):
            # ---------- persistent SBUF ----------
            h = res.tile([128, NSHp], f32, name="h_act")   # 0:H h, 96 score,
            agg = res.tile([H, NSHp], f32, name="agg")     # 97 ones
            dinv_bc = res.tile([H, NSHp], f32, name="dinv_bc")
            dinv_pp = res.tile([128, NT], f32, name="dinv_pp")
            iota16 = res.tile([128, 128], bf16, name="iota16")
            ident = res.tile([128, 128], f32, name="ident")
            idxg = res.tile([128, (slots[0] + slots[1]) // 16], i16,
                            name="idxg")
            dlg = res.tile([128, nblk[0] + nblk[1]], bf16, name="dlg")
            oh_sp = res.tile([128, K * 128], bf16, name="oh_sp")
            oht_sp = res.tile([128, K * 128], bf16, name="oht_sp")
            wproj = res.tile([cfg.IN, H], f32, name="wproj")
            w1 = res.tile([H, H], f32, name="w1")
            w2 = res.tile([H, H], f32, name="w2")
            wa1 = res.tile([H, AH], f32, name="wa1")
            wa2 = res.tile([AH, 1], f32, name="wa2")
            wm1 = res.tile([H, H], f32, name="wm1")
            wm2 = res.tile([H, OD], f32, name="wm2")
            pft = res.tile([H, 16], f32, name="pft")
            ab1 = res.tile([AH, 1], f32, name="ab1")
            ab2 = res.tile([1, 1], f32, name="ab2")
            mb2 = res.tile([OD, 1], f32, name="mb2")
            sq = res.tile([H, 512], f32, name="sq")
            vec = res.tile([H, 8], f32, name="vec")
            Rst = res.tile([128, NT * 98], bf16, name="Rst")  # spot rows
            Evec = res.tile([128, NT], bf16, name="Evec")     # exp(score)
            DNcol = res.tile([128, ST], bf16, name="DNcol")   # 1/denom cols
            DNf = res.tile([128, ST], f32, name="DNf")
            spot_fm = res.tile([H, ST * 128], f32, name="spot_fm")

            for t_, s_ in ((iota16, iota16_in), (ident, ident_in),
                           (idxg, idx_gcn), (dlg, dl_gcn),
                           (oh_sp, oh_sp_in), (oht_sp, oht_sp_in),
                           (wproj, projW), (w1, W1), (w2, W2),
                           (wa1, attnW1), (wa2, attnW2), (wm1, mlpW1),
                           (wm2, mlpW2), (pft, pf), (ab1, attn_b1),
                           (ab2, attn_b2), (mb2, mlp_b2)):
                nc.sync.dma_start(t_[:], s_[:])

            # dinv = rsqrt(deg); pad cols get deg=1e30 -> dinv ~ 0
            nc.sync.dma_start(dinv_pp[:], deg_pp[:])
            nc.scalar.activation(dinv_pp[:], dinv_pp[:],
                                 mybir.ActivationFunctionType.Sqrt)
            nc.vector.reciprocal(dinv_pp[:], dinv_pp[:])
            nc.sync.dma_start(dinv_bc[:], deg_bc[:])
            nc.scalar.activation(dinv_bc[:], dinv_bc[:],
                                 mybir.ActivationFunctionType.Sqrt)
            nc.vector.reciprocal(dinv_bc[:], dinv_bc[:])

            tbl_own = [dram.tile([NSHp, TROWB], bf16, name=f"tblo{i}")
                       for i in range(2)]
            tbl_full = [dram.tile([N_CORES * NSHp, TROWB], bf16,
                                  addr_space="Shared", name=f"tblf{i}")
                        for i in range(2)]
            st_in = [dram.tile([H, 2], f32, name=f"sti{i}") for i in range(4)]
            st_out = [dram.tile([H, 2], f32, addr_space="Shared",
                                name=f"sto{i}") for i in range(4)]

            NCHUNK = (NSHp + 511) // 512

            def csz(ci):
                return min(512, NSHp - ci * 512)

            def graph_norm_relu(dst_ap, u_ap, width, n_total, stats_idx,
                                pre_b_col, gn_cols):
                """dst = relu(S*u + B) with GN stats over u[:, :width]."""
                s1 = vec[:, 0:1]
                nc.vector.tensor_reduce(s1, u_ap[:, :width],
                                        mybir.AxisListType.X,
                                        mybir.AluOpType.add)
                nch = (width + 511) // 512
                s2p = res.tile([H, nch], f32, name=f"s2p{stats_idx}")
                for ci in range(nch):
                    w_ = min(512, width - ci * 512)
                    nc.scalar.activation(
                        sq[:, :w_], u_ap[:, ci * 512:ci * 512 + w_],
                        mybir.ActivationFunctionType.Square,
                        accum_out=s2p[:, ci:ci + 1])
                nc.vector.tensor_reduce(vec[:, 1:2], s2p[:],
                                        mybir.AxisListType.X,
                                        mybir.AluOpType.add)
                stv = stg.tile([H, 2], f32, name=f"stv{stats_idx}")
                nc.vector.tensor_copy(stv[:], vec[:, 0:2])
                nc.sync.dma_start(st_in[stats_idx][:], stv[:])
                nc.gpsimd.collective_compute(
                    "AllReduce", mybir.AluOpType.add,
                    replica_groups=[list(range(N_CORES))],
                    ins=[st_in[stats_idx][:].opt()],
                    outs=[st_out[stats_idx][:].opt()])
                stt = stg.tile([H, 2], f32, name=f"stt{stats_idx}")
                nc.sync.dma_start(stt[:], st_out[stats_idx][:])
                gw = pft[:, gn_cols[0]:gn_cols[0] + 1]
                gb = pft[:, gn_cols[1]:gn_cols[1] + 1]
                ga = pft[:, gn_cols[2]:gn_cols[2] + 1]
                mean = vec[:, 2:3]
                ex2 = vec[:, 3:4]
                inv_n = 1.0 / float(n_total)
                nc.vector.tensor_scalar(mean, stt[:, 0:1], inv_n, None,
                                        mybir.AluOpType.mult)
                nc.vector.tensor_scalar(ex2, stt[:, 1:2], inv_n, None,
                                        mybir.AluOpType.mult)
                if pre_b_col is not None:
                    c_ = pft[:, pre_b_col:pre_b_col + 1]
                    t0 = vec[:, 4:5]
                    nc.vector.tensor_tensor(t0, c_, mean, mybir.AluOpType.mult)
                    nc.vector.tensor_scalar(t0, t0, 2.0, None,
                                            mybir.AluOpType.mult)
                    nc.vector.tensor_tensor(ex2, ex2, t0, mybir.AluOpType.add)
                    t1 = vec[:, 5:6]
                    nc.vector.tensor_tensor(t1, c_, c_, mybir.AluOpType.mult)
                    nc.vector.tensor_tensor(ex2, ex2, t1, mybir.AluOpType.add)
                    nc.vector.tensor_tensor(mean, mean, c_, mybir.AluOpType.add)
                m2 = vec[:, 4:5]
                nc.vector.tensor_tensor(m2, mean, mean, mybir.AluOpType.mult)
                a2 = vec[:, 5:6]
                nc.vector.tensor_scalar(a2, ga, -1.0, 2.0,
                                        mybir.AluOpType.mult,
                                        mybir.AluOpType.add)
                nc.vector.tensor_tensor(a2, a2, ga, mybir.AluOpType.mult)
                nc.vector.tensor_tensor(m2, m2, a2, mybir.AluOpType.mult)
                var = vec[:, 6:7]
                nc.vector.tensor_tensor(var, ex2, m2,
                                        mybir.AluOpType.subtract)
                nc.vector.tensor_scalar(var, var, float(cfg.EPS), None,
                                        mybir.AluOpType.add)
                nc.scalar.activation(var, var,
                                     mybir.ActivationFunctionType.Sqrt)
                nc.vector.reciprocal(var, var)
                Sg = vec[:, 4:5]
                nc.vector.tensor_tensor(Sg, gw, var, mybir.AluOpType.mult)
                Bg = vec[:, 5:6]
                nc.vector.tensor_tensor(Bg, Sg, ga, mybir.AluOpType.mult)
                nc.vector.tensor_tensor(Bg, Bg, mean, mybir.AluOpType.mult)
                nc.vector.tensor_tensor(Bg, gb, Bg, mybir.AluOpType.subtract)
                if pre_b_col is not None:
                    c_ = pft[:, pre_b_col:pre_b_col + 1]
                    t0 = vec[:, 6:7]
                    nc.vector.tensor_tensor(t0, Sg, c_, mybir.AluOpType.mult)
                    nc.vector.tensor_tensor(Bg, Bg, t0, mybir.AluOpType.add)
                nc.scalar.activation(dst_ap, u_ap,
                                     mybir.ActivationFunctionType.Relu,
                                     bias=Bg, scale=Sg)

            # ================= proj layer =================
            nc.sync.dma_start(h[:cfg.IN, :], xT[:])
            for ci in range(NCHUNK):
                w_ = csz(ci)
                ps = mmp.tile([H, 512], f32, name=f"pj{ci}", tag="mm")
                nc.tensor.matmul(ps[:, :w_], wproj[:],
                                 h[:cfg.IN, ci * 512:ci * 512 + w_],
                                 start=True, stop=True)
                nc.vector.tensor_copy(agg[:, ci * 512:ci * 512 + w_],
                                      ps[:, :w_])
            graph_norm_relu(h[:H, :], agg[:], NSHp, cfg.N, 0, 0, (1, 2, 3))
            if DEBUG:
                nc.sync.dma_start(dbg_h0[:], h[:H, :])

            # ================= GCN layers =================
            gsems = [nc.alloc_semaphore(f"gdma{q}") for q in range(4)]  # 1..3 used
            GBUF = 3                         # gather tiles preppable ahead
            qfired = {1: 0, 2: 0, 3: 0}      # calls fired per queue (x16 sem)
            qtarget = {}                     # idx_call -> (queue, sem target)
            for li, (Wt, b_col, gn_cols) in enumerate(
                    ((w1, 4, (5, 6, 7)), (w2, 8, (9, 10, 11)))):
                # table rows t' = dinv * (h @ W), bf16 [NSHp, 128]
                for t in range(NT):
                    ps = mmp.tile([128, H], f32, name=f"tb{li}_{t}", tag="mm")
                    nc.tensor.matmul(ps[:, :], h[:H, t * 128:(t + 1) * 128],
                                     Wt[:], start=True, stop=True)
                    sg = stg.tile([128, TROWB], bf16, name=f"ts{li}_{t}",
                                  tag="tstg")
                    nc.vector.memset(sg[:, H:], 0.0)
                    nc.vector.tensor_scalar(sg[:, :H], ps[:, :],
                                            dinv_pp[:, t:t + 1], None,
                                            mybir.AluOpType.mult)
                    nc.sync.dma_start(
                        tbl_own[li][t * 128:(t + 1) * 128, :], sg[:])
                nc.gpsimd.collective_compute(
                    "AllGather", mybir.AluOpType.bypass,
                    replica_groups=[list(range(N_CORES))],
                    ins=[tbl_own[li][:].opt()], outs=[tbl_full[li][:].opt()])
                # self-loop: agg = (W^T h) * dinv_bc
                for ci in range(NCHUNK):
                    w_ = csz(ci)
                    ps = mmp.tile([H, 512], f32, name=f"sf{li}_{ci}", tag="mm")
                    nc.tensor.matmul(ps[:, :w_], Wt[:],
                                     h[:H, ci * 512:ci * 512 + w_],
                                     start=True, stop=True)
                    nc.vector.tensor_tensor(
                        agg[:, ci * 512:ci * 512 + w_], ps[:, :w_],
                        dinv_bc[:, ci * 512:ci * 512 + w_],
                        mybir.AluOpType.mult)
                # gather + scatter, two phases (src halves).
                # dma_gather runs prepare_only: descriptor generation starts
                # during the AllGather (no table dependency); the trigger
                # carries the table read and fires as soon as it lands. The
                # first GBUF preps are emitted back to back so they fill the
                # collective window; after that each prep is paired with its
                # trigger (WAW on the gather tiles paces the pipeline).
                allcalls = []
                for hph in (0, 1):
                    tile_of = np.repeat(np.arange(NT), S[:, hph])
                    for k, (o, n) in enumerate(gcalls[hph]):
                        allcalls.append((hph, k, o, n, tile_of))
                qn = 1
                gtiles = {}
                st_cons = dict(open_ps=None, open_t=-1, prev_hph=0)

                def consume(ic):
                    hph, k, o, n, tile_of = allcalls[ic]
                    nb = n // 128
                    blk0 = 0 if hph == 0 else nblk[0]
                    if hph != st_cons['prev_hph']:
                        st_cons['open_t'] = -1
                        st_cons['prev_hph'] = hph
                    g = gtiles.pop(ic)
                    # tile's deferred-dep tracking does not model the DMA
                    # completion of a prepared gather; wait on the
                    # descriptor-embedded sem explicitly (16 engines x +1)
                    wq, wt = qtarget[ic]
                    nc.tensor.wait_ge(gsems[wq], 16 * wt)
                    oh = ohp.tile([128, CB, 128], bf16,
                                  name=f"oh{li}_{hph}_{k}", tag="oh")
                    dlsl = dlg[:, blk0 + o // 128: blk0 + (o + n) // 128]
                    nc.vector.tensor_tensor(
                        oh[:, :nb, :],
                        iota16[:].unsqueeze(1).broadcast_to([128, nb, 128]),
                        dlsl.unsqueeze(2).broadcast_to([128, nb, 128]),
                        mybir.AluOpType.is_equal)
                    for j in range(nb):
                        b = o // 128 + j
                        t = int(tile_of[b])
                        if t != st_cons['open_t']:
                            st_cons['open_ps'] = scp.tile(
                                [H, 128], f32, name=f"sc{li}_{hph}_{b}",
                                tag="sc")
                            st_cons['open_t'] = t
                            first = True
                        else:
                            first = False
                        last = (b + 1 == len(tile_of)) or \
                               (tile_of[b + 1] != t)
                        nc.tensor.matmul(st_cons['open_ps'][:], g[:, j, :H],
                                         oh[:, j, :],
                                         start=first, stop=last)
                        if last:
                            nc.vector.tensor_tensor(
                                agg[:, t * 128:(t + 1) * 128],
                                agg[:, t * 128:(t + 1) * 128],
                                st_cons['open_ps'][:], mybir.AluOpType.add)

                pend = []
                for idx_call, (hph, k, o, n, tile_of) in enumerate(allcalls):
                    nb = n // 128
                    tview = tbl_full[li][hph * cfg.HALF:
                                         hph * cfg.HALF + cfg.HALF, :]
                    col0 = 0 if hph == 0 else slots[0] // 16
                    g = gat.tile([128, CB, TROWB], bf16,
                                 name=f"g{li}_{hph}_{k}", tag="gat")
                    gtiles[idx_call] = g
                    nc.gpsimd.dma_gather(
                        g[:, :nb, :], tview,
                        idxg[:, col0 + o // 16: col0 + (o + n) // 16],
                        n, n, TROWB, single_packet=False, queue_num=qn,
                        prepare_only=True, sem=gsems[qn])
                    qfired[qn] += 1
                    qtarget[idx_call] = (qn, qfired[qn])
                    if idx_call < GBUF - 1:
                        pend.append(qn)
                    else:
                        if pend:
                            # the deferred table-read dep does not reach the
                            # trigger; gate the GpSimd queue on the AllGather
                            # with a tiny read of its output before firing
                            tg = stg.tile([1, TROWB], bf16,
                                          name=f"tg{li}", tag="tgate")
                            nc.gpsimd.dma_start(tg[:], tbl_full[li][0:1, :])
                            for q_ in pend:
                                nc.gpsimd.trigger_dma(count=None,
                                                      queue_num=q_)
                            pend = []
                        nc.gpsimd.trigger_dma(count=None, queue_num=qn)
                        consume(idx_call - (GBUF - 1))
                    qn = 1 + (qn % 3)
                for q_ in pend:
                    nc.gpsimd.trigger_dma(count=None, queue_num=q_)
                for ic in range(len(allcalls) - (GBUF - 1), len(allcalls)):
                    if ic >= 0:
                        consume(ic)
                # u = agg * dinv (per dst node)
                nc.vector.tensor_tensor(agg[:], agg[:], dinv_bc[:],
                                        mybir.AluOpType.mult)
                if DEBUG:
                    nc.sync.dma_start(dbg_ag[li][:], agg[:])
                graph_norm_relu(h[:H, :], agg[:], NSHp, cfg.N,
                                1 + li, b_col, gn_cols)
                if DEBUG:
                    nc.sync.dma_start((dbg_h1 if li == 0 else dbg_h2)[:],
                                      h[:H, :])

            # ================= attention scores =================
            # rows 96:98 <- 1.0 first; attn then overwrites row 96 with the
            # score, leaving row 97 as the ones row for the spot denominator
            # (a lone write at partition 97 breaks the 32-partition alignment
            # rule, so both rows are set together).
            nc.vector.memset(h[96:98, :], 1.0)
            for ci in range(NCHUNK):
                w_ = csz(ci)
                ps = mmp.tile([AH, 512], f32, name=f"at{ci}", tag="mm")
                nc.tensor.matmul(ps[:, :w_], wa1[:],
                                 h[:H, ci * 512:ci * 512 + w_],
                                 start=True, stop=True)
                uc = stg.tile([AH, 512], f32, name=f"uat{ci}", tag="uat")
                nc.scalar.activation(uc[:, :w_], ps[:, :w_],
                                     mybir.ActivationFunctionType.Relu,
                                     bias=ab1[:])
                ps2 = mmp.tile([1, 512], f32, name=f"sc2{ci}", tag="mm1")
                nc.tensor.matmul(ps2[:, :w_], wa2[:], uc[:, :w_],
                                 start=True, stop=True)
                nc.vector.tensor_scalar(h[H:H + 1, ci * 512:ci * 512 + w_],
                                        ps2[:, :w_], ab2[:],
                                        None, mybir.AluOpType.add)
            if DEBUG:
                nc.sync.dma_start(dbg_sc[:], h[H:H + 1, :])

            # ================= spot aggregation (core-local) ==============
            # R_g = transpose(h[0:98, tile]) node rows [h | score | 1];
            # e = exp(score); den[s] = OH^T e (column); dn = OHT^T (1/den);
            # spot_fm[:, sg] = sum_g (R_g * e * dn)^T OH_{g,sg}
            for g_ in range(NT):
                pt = mmp.tile([128, 98], f32, name=f"tr{g_}", tag="tpose")
                nc.tensor.transpose(pt[:, :98], h[:98, g_ * 128:(g_ + 1) * 128],
                                    ident[:98, :98])
                nc.vector.tensor_copy(Rst[:, g_ * 98:(g_ + 1) * 98], pt[:])
                nc.scalar.activation(Evec[:, g_:g_ + 1], pt[:, 96:97],
                                     mybir.ActivationFunctionType.Exp)
            # denominators per spot group: den[s] = sum_n e_n OH[n, s]
            for sg in range(ST):
                ks = [k for k, (g_, s_) in enumerate(spairs) if s_ == sg]
                dps = mmp.tile([128, 1], f32, name=f"dn{sg}", tag="mm1")
                for i, k in enumerate(ks):
                    g_ = spairs[k][0]
                    nc.tensor.matmul(dps[:], oh_sp[:, k * 128:(k + 1) * 128],
                                     Evec[:, g_:g_ + 1],
                                     start=(i == 0), stop=(i == len(ks) - 1))
                nc.vector.tensor_scalar(DNf[:, sg:sg + 1], dps[:], 1e-30,
                                        None, mybir.AluOpType.max)
            nc.vector.reciprocal(DNf[:], DNf[:])
            nc.vector.tensor_copy(DNcol[:], DNf[:])
            if DEBUG:
                nc.sync.dma_start(dbg_den[:], DNf[:])
            # weighted rows and numerators
            wcol = res.tile([128, NT], f32, name="wcol")
            for g_ in range(NT):
                ks = [k for k, (gg, s_) in enumerate(spairs) if gg == g_]
                dn = mmp.tile([128, 1], f32, name=f"dnn{g_}", tag="mm1")
                for i, k in enumerate(ks):
                    sg = spairs[k][1]
                    nc.tensor.matmul(dn[:], oht_sp[:, k * 128:(k + 1) * 128],
                                     DNcol[:, sg:sg + 1],
                                     start=(i == 0), stop=(i == len(ks) - 1))
                nc.vector.tensor_tensor(wcol[:, g_:g_ + 1],
                                        Evec[:, g_:g_ + 1], dn[:],
                                        mybir.AluOpType.mult)
            Rw = res.tile([128, NT * 98], bf16, name="Rw")
            for g_ in range(NT):
                nc.vector.tensor_scalar(Rw[:, g_ * 98:(g_ + 1) * 98],
                                        Rst[:, g_ * 98:(g_ + 1) * 98],
                                        wcol[:, g_:g_ + 1], None,
                                        mybir.AluOpType.mult)
            for sg in range(ST):
                ks = [k for k, (g_, s_) in enumerate(spairs) if s_ == sg]
                sps = mmp.tile([98, 128], f32, name=f"sp{sg}", tag="tpose")
                for i, k in enumerate(ks):
                    g_ = spairs[k][0]
                    nc.tensor.matmul(sps[:], Rw[:, g_ * 98:(g_ + 1) * 98],
                                     oh_sp[:, k * 128:(k + 1) * 128],
                                     start=(i == 0), stop=(i == len(ks) - 1))
                nc.vector.tensor_copy(spot_fm[:, sg * 128:(sg + 1) * 128],
                                      sps[:H, :])
            if DEBUG:
                nc.sync.dma_start(dbg_sp[:], spot_fm[:])

            # ================= MLP head =================
            um = res.tile([H, ST * 128], f32, name="um")
            for ci in range((ST * 128 + 511) // 512):
                w_ = min(512, ST * 128 - ci * 512)
                ps = mmp.tile([H, 512], f32, name=f"m1{ci}", tag="mm")
                nc.tensor.matmul(ps[:, :w_], wm1[:],
                                 spot_fm[:, ci * 512:ci * 512 + w_],
                                 start=True, stop=True)
                nc.vector.tensor_copy(um[:, ci * 512:ci * 512 + w_],
                                      ps[:, :w_])
            graph_norm_relu(um[:], um[:], SSH, cfg.S, 3, 12, (13, 14, 15))
            zo = res.tile([OD, ST * 128], f32, name="zo")
            for ci in range((ST * 128 + 511) // 512):
                w_ = min(512, ST * 128 - ci * 512)
                ps = mmp.tile([OD, 512], f32, name=f"m2{ci}", tag="mm")
                nc.tensor.matmul(ps[:, :w_], wm2[:],
                                 um[:, ci * 512:ci * 512 + w_],
                                 start=True, stop=True)
                nc.vector.tensor_scalar(zo[:, ci * 512:ci * 512 + w_],
                                        ps[:, :w_], mb2[:], None,
                                        mybir.AluOpType.add)
            for gi in range(ST):
                n_ = min(128, SSH - gi * 128)
                if n_ <= 0:
                    break
                ps = mmp.tile([128, OD], f32, name=f"ot{gi}", tag="tpose")
                nc.tensor.transpose(ps[:, :], zo[:, gi * 128:(gi + 1) * 128],
                                    ident[:OD, :OD])
                sg = stg.tile([128, OD], f32, name=f"os{gi}", tag="ostg")
                nc.vector.tensor_copy(sg[:], ps[:])
                nc.sync.dma_start(out[gi * 128:gi * 128 + n_, :], sg[:n_, :])

    nc.compile()
    return nc


_CACHE = {}


def _build_inputs(cfg, inputs, perm, starts, counts, idx_w, dl_w,
                  oh_l, oht_l, deg_new):
    f = np.float32
    import ml_dtypes
    x = np.asarray(inputs['x'], f)
    NSHp, NT, H = cfg.NSHp, cfg.NT, cfg.H

    def col(v):
        return np.asarray(v, f).reshape(-1, 1)

    pf = np.zeros((H, 16), f)
    for i, k in enumerate(['proj_b', 'gn0_w', 'gn0_b', 'gn0_a',
                           'gcn1_b', 'gn1_w', 'gn1_b', 'gn1_a',
                           'gcn2_b', 'gn2_w', 'gn2_b', 'gn2_a',
                           'mlp_b1', 'mlp_gn_w', 'mlp_gn_b', 'mlp_gn_a']):
        pf[:, i] = np.asarray(inputs[k], f)
    iota = np.broadcast_to(np.arange(128, dtype=f), (128, 128)).copy()
    iota16 = iota.astype(ml_dtypes.bfloat16)
    ident = np.eye(128, dtype=f)
    in_maps = []
    for c in range(N_CORES):
        cnt = int(counts[c])
        own = perm[starts[c]:starts[c] + cnt]       # old ids, new order
        xc = np.zeros((cfg.IN, NSHp), f)
        xc[:, :cnt] = x[own].T
        deg_own = np.full(NSHp, 1e30, f)
        deg_own[:cnt] = deg_new[starts[c]:starts[c] + cnt]
        dpp = np.full((128, NT), 1e30, f)
        for t in range(NT):
            dpp[:, t] = deg_own[t * 128:(t + 1) * 128]
        dbc = np.broadcast_to(deg_own[None, :], (H, NSHp)).copy()
        in_maps.append({
            'xT': xc, 'deg_pp': dpp, 'deg_bc': dbc,
            'iota16_in': iota16, 'ident_in': ident,
            'idx_gcn': idx_w[c],
            'dl_gcn': dl_w[c].astype(ml_dtypes.bfloat16),
            'oh_sp_in': oh_l[c].astype(ml_dtypes.bfloat16),
            'oht_sp_in': oht_l[c].astype(ml_dtypes.bfloat16),
            'projW': np.asarray(inputs['proj_W'], f),
            'W1': np.asarray(inputs['gcn1_W'], f),
            'W2': np.asarray(inputs['gcn2_W'], f),
            'attnW1': np.asarray(inputs['attn_W1'], f),
            'attnW2': np.asarray(inputs['attn_W2'], f),
            'mlpW1': np.asarray(inputs['mlp_W1'], f),
            'mlpW2': np.asarray(inputs['mlp_W2'], f),
            'pf': pf,
            'attn_b1': col(inputs['attn_b1']),
            'attn_b2': col(inputs['attn_b2']),
            'mlp_b2': col(inputs['mlp_b2']),
        })
    return in_maps


def kernel(**inputs):
    from concourse import bass_utils
    cfg = Cfg(n_nodes=int(np.asarray(inputs['x']).shape[0]),
              n_edges=int(np.asarray(inputs['edge_index']).shape[1]),
              in_dim=int(np.asarray(inputs['x']).shape[1]),
              hid=int(np.asarray(inputs['proj_W']).shape[1]),
              attn_hid=int(np.asarray(inputs['attn_W1']).shape[1]),
              out_dim=int(np.asarray(inputs['mlp_W2']).shape[1]),
              n_spots=int(inputs['num_spots']))
    ei = np.asarray(inputs['edge_index']).astype(np.int64)
    cts = np.asarray(inputs['cell_to_spot']).astype(np.int64)
    src, dst = ei[0], ei[1]

    perm, starts, counts, spot_sorted = _relabel(cfg, cts)
    inv = np.empty(cfg.N, np.int64)
    inv[perm] = np.arange(cfg.N)
    src_new, dst_new = inv[src], inv[dst]
    # node -> core and global table row id
    node_core = np.searchsorted(starts, np.arange(cfg.N), side='right') - 1
    # starts from _relabel are in new-id space: node i (new) on core c iff
    # starts[c] <= i < starts[c]+counts[c]
    loc = np.arange(cfg.N) - starts[node_core]
    gid = node_core * cfg.NSHp + loc
    src_gid = gid[src_new]
    deg_full = (np.bincount(dst_new, minlength=cfg.N) + 1).astype(np.float32)

    idx_w, dl_w, emeta = _prep_edges(cfg, src_gid, dst_new, node_core, starts)
    spairs, oh_l, oht_l = _prep_spot_oh(cfg, spot_sorted, starts, counts)

    key = (cfg.N, cfg.E, cfg.NSHp, tuple(emeta['nblk']), len(spairs))
    if key not in _CACHE:
        _CACHE[key] = build_program(cfg, emeta, spairs)
    nc = _CACHE[key]

    in_maps = _build_inputs(cfg, inputs, perm, starts, counts, idx_w, dl_w,
                            oh_l, oht_l, deg_full)
    res = bass_utils.run_bass_kernel_spmd(
        nc, in_maps, core_ids=list(range(N_CORES)),
        trace=os.environ.get('KERNEL_TRACE', '0') == '1',
        tmpdir=os.environ.get('KERNEL_TMPD'))
    if os.environ.get('KERNEL_TRACE', '0') == '1':
        print('HW exec time:', res.exec_time_ns, 'ns')
    out = np.concatenate([res.results[c]['out'] for c in range(N_CORES)],
                         axis=0)
    return out.astype(np.float32)


# revision 20
# speedup vs baseline: 1.7095x; 1.7095x over previous
"""GCN pipeline (proj + 2x GCNConv + GraphNorm + spot-softmax aggregation +
MLP head) on 8 trn2 NeuronCores via Bass/Tile.

Nodes are relabeled spot-sorted on the host so core c owns spots
[c*SSH,(c+1)*SSH) and exactly the nodes belonging to those spots (padded to
NSHp). GCN tables are bf16 [*, 128] rows (256B), AllGathered, then dst-sorted
edges are gathered by src (dma_gather) and scatter-accumulated per dst tile
with feature-major one-hot matmuls (psum [H, 128dst] adds straight into agg).
The spot softmax-aggregation is core-local: static host-built one-hot
matrices turn it into ~50 small matmuls with no gathers.
"""
import sys, os
sys.path.insert(0, '/opt/trn_rl_repo')
import numpy as np

N_CORES = 8
HALF_BITS = 15


class Cfg:
    def __init__(self, n_nodes=50000, n_edges=800000, in_dim=128, hid=96,
                 attn_hid=32, out_dim=16, n_spots=5000, eps=1e-5):
        assert n_spots % N_CORES == 0
        self.N, self.E, self.IN, self.H = n_nodes, n_edges, in_dim, hid
        self.AH, self.OD, self.S, self.EPS = attn_hid, out_dim, n_spots, eps
        self.SSH = n_spots // N_CORES
        self.ST = (self.SSH + 127) // 128           # spot groups per core
        self.TROWB = 128                            # table row bf16 (256B)
        self.CALL = 4096                            # gather slots per call
        # filled after relabel:
        self.NSHp = None
        self.NT = None
        self.HALF = None


def _relabel(cfg, cts):
    """Spot-sort nodes; returns perm (new->old), per-core start/count, NSHp."""
    perm = np.argsort(cts, kind='stable')
    spot_sorted = cts[perm]
    # core boundaries: first node whose spot >= c*SSH
    starts = np.searchsorted(spot_sorted, np.arange(N_CORES + 1) * cfg.SSH)
    counts = np.diff(starts)
    NSHp = int(((counts.max() + 127) // 128) * 128)
    cfg.NSHp = NSHp
    cfg.NT = NSHp // 128
    cfg.HALF = (N_CORES // 2) * NSHp
    assert cfg.HALF < (1 << HALF_BITS), cfg.HALF
    return perm, starts[:-1], counts, spot_sorted


def _wrap_idx(flat):
    n = len(flat)
    assert n % 16 == 0
    w = flat.reshape(n // 16, 16).T.astype(np.int16)
    return np.tile(w, (8, 1))


def _prep_edges(cfg, src_gid, dst_new, node_core, starts):
    """Per-core dst-sorted edge structure, uniform across cores for SPMD.

    src_gid: per-edge global table row id (core*NSHp + local).
    dst_new: per-edge new (relabeled) dst node id.
    Block order per core: phase A (src_gid < HALF): tiles 0..NT-1, S[t,0]
    blocks each; then phase B likewise with S[t,1].
    """
    NSHp, NT, HALF = cfg.NSHp, cfg.NT, cfg.HALF
    core_of = node_core[dst_new]
    per_core = []
    cnt_all = np.zeros((N_CORES, NT, 2), np.int64)
    for c in range(N_CORES):
        m = core_of == c
        s_c = src_gid[m]
        d_c = dst_new[m] - starts[c]
        t_c = d_c // 128
        h_c = (s_c >= HALF).astype(np.int64)
        key = t_c * 2 + h_c
        order = np.argsort(key, kind='stable')
        per_core.append((s_c[order], d_c[order], key[order]))
        cnt_all[c] = np.bincount(key, minlength=NT * 2).reshape(NT, 2)
    S = (cnt_all.max(axis=0) + 127) // 128
    nblk = [int(S[:, h].sum()) for h in (0, 1)]
    slots = [n * 128 for n in nblk]
    b0 = np.zeros((NT, 2), np.int64)
    for h in (0, 1):
        b0[:, h] = np.cumsum(S[:, h]) - S[:, h]
    idx_w, dl_w = [], []
    for c in range(N_CORES):
        s_c, d_c, key = per_core[c]
        bounds = np.searchsorted(key, np.arange(NT * 2 + 1))
        idx_flat = np.zeros(slots[0] + slots[1], np.int64)
        dl_flat = -np.ones(slots[0] + slots[1], np.float32)
        for t in range(NT):
            for h in (0, 1):
                lo, hi = bounds[t * 2 + h], bounds[t * 2 + h + 1]
                n = hi - lo
                off = (0 if h == 0 else slots[0]) + int(b0[t, h]) * 128
                ss = s_c[lo:hi] - (0 if h == 0 else HALF)
                idx_flat[off:off + n] = ss
                dl_flat[off:off + n] = (d_c[lo:hi] % 128).astype(np.float32)
        idx_w.append(_wrap_idx(idx_flat))
        dl_w.append(np.ascontiguousarray(
            dl_flat.reshape(-1, 128).T))            # [128, nblk_tot]
    meta = dict(S=S, b0=b0, nblk=nblk, slots=slots)
    return idx_w, dl_w, meta


def _prep_spot_oh(cfg, spot_sorted, starts, counts):
    """Static one-hot matrices for the core-local spot aggregation.

    Returns the uniform (g, sg) pair list and per-core OH / OHT stacks:
    OH[k][128 node, 128 spot] for pair k=(g,sg); OHT[k] its transpose.
    """
    NT, ST, SSH = cfg.NT, cfg.ST, cfg.SSH
    pairs = set()
    percore_sp = []
    for c in range(N_CORES):
        sp_loc = spot_sorted[starts[c]:starts[c] + counts[c]] - c * SSH
        percore_sp.append(sp_loc)
        for g in range(NT):
            lo, hi = g * 128, min((g + 1) * 128, counts[c])
            if lo >= hi:
                continue
            for sg in range(sp_loc[lo] // 128, sp_loc[hi - 1] // 128 + 1):
                pairs.add((g, int(sg)))
    pairs = sorted(pairs)
    K = len(pairs)
    oh_l, oht_l = [], []
    for c in range(N_CORES):
        sp_loc = percore_sp[c]
        oh = np.zeros((128, K, 128), np.float32)
        for k, (g, sg) in enumerate(pairs):
            lo, hi = g * 128, min((g + 1) * 128, counts[c])
            for p in range(lo, hi):
                s = sp_loc[p - lo + lo]  # == sp_loc[p]
                if sg * 128 <= s < (sg + 1) * 128:
                    oh[p - lo, k, s - sg * 128] = 1.0
        oht = np.ascontiguousarray(oh.transpose(2, 1, 0))
        oh_l.append(oh.reshape(128, K * 128))
        oht_l.append(oht.reshape(128, K * 128))
    return pairs, oh_l, oht_l


def _calls(total_slots, call):
    out = []
    o = 0
    while o < total_slots:
        n = min(call, total_slots - o)
        out.append((o, n))
        o += n
    return out


def build_program(cfg, emeta, spairs):
    from concourse import bacc, mybir, tile

    f32, i16, bf16 = mybir.dt.float32, mybir.dt.int16, mybir.dt.bfloat16
    H, AH, OD = cfg.H, cfg.AH, cfg.OD
    NSHp, NT, SSH, ST = cfg.NSHp, cfg.NT, cfg.SSH, cfg.ST
    TROWB = cfg.TROWB
    S, nblk, slots = emeta['S'], emeta['nblk'], emeta['slots']
    K = len(spairs)

    nc = bacc.Bacc("TRN2", target_bir_lowering=False, debug=False,
                   num_devices=N_CORES, num_swdge_queues=4)

    def din(name, shape, dt=f32):
        return nc.dram_tensor(name, shape, dt, kind="ExternalInput")

    xT = din("xT", [cfg.IN, NSHp])
    deg_pp = din("deg_pp", [128, NT])
    deg_bc = din("deg_bc", [H, NSHp])
    iota16_in = din("iota16_in", [128, 128], bf16)
    ident_in = din("ident_in", [128, 128])
    idx_gcn = din("idx_gcn", [128, (slots[0] + slots[1]) // 16], i16)
    dl_gcn = din("dl_gcn", [128, nblk[0] + nblk[1]], bf16)
    oh_sp_in = din("oh_sp_in", [128, K * 128], bf16)
    oht_sp_in = din("oht_sp_in", [128, K * 128], bf16)
    projW = din("projW", [cfg.IN, H])
    W1, W2 = din("W1", [H, H]), din("W2", [H, H])
    attnW1, attnW2 = din("attnW1", [H, AH]), din("attnW2", [AH, 1])
    mlpW1, mlpW2 = din("mlpW1", [H, H]), din("mlpW2", [H, OD])
    pf = din("pf", [H, 16])
    attn_b1 = din("attn_b1", [AH, 1])
    attn_b2 = din("attn_b2", [1, 1])
    mlp_b2 = din("mlp_b2", [OD, 1])
    out = nc.dram_tensor("out", [SSH, OD], f32, kind="ExternalOutput")
    DEBUG = os.environ.get('KERNEL_DEBUG', '0') == '1'
    if DEBUG:
        dbg_h0 = nc.dram_tensor("dbg_h0", [H, NSHp], f32, kind="ExternalOutput")
        dbg_h1 = nc.dram_tensor("dbg_h1", [H, NSHp], f32, kind="ExternalOutput")
        dbg_h2 = nc.dram_tensor("dbg_h2", [H, NSHp], f32, kind="ExternalOutput")
        dbg_sc = nc.dram_tensor("dbg_sc", [1, NSHp], f32, kind="ExternalOutput")
        dbg_sp = nc.dram_tensor("dbg_sp", [H, ST * 128], f32,
                                kind="ExternalOutput")
        dbg_den = nc.dram_tensor("dbg_den", [128, ST], f32,
                                 kind="ExternalOutput")
        dbg_ag = [nc.dram_tensor(f"dbg_ag{l}", [H, NSHp], f32,
                                 kind="ExternalOutput") for l in range(2)]

    gcalls = [_calls(slots[0], cfg.CALL), _calls(slots[1], cfg.CALL)]
    CB = cfg.CALL // 128                     # blocks per full call

    with tile.TileContext(nc) as tc:
        with (
            tc.tile_pool(name="res", bufs=1) as res,
            tc.tile_pool(name="gat", bufs=3) as gat,
            tc.tile_pool(name="ohp", bufs=2) as ohp,
            tc.tile_pool(name="stg", bufs=3) as stg,
            tc.tile_pool(name="spp", bufs=2) as spp,
            tc.tile_pool(name="mmp", bufs=2, space="PSUM") as mmp,
            tc.tile_pool(name="scp", bufs=2, space="PSUM") as scp,
            tc.tile_pool(name="dram", bufs=1, space="DRAM") as dram,
        ):
            # ---------- persistent SBUF ----------
            h = res.tile([128, NSHp], f32, name="h_act")   # 0:H h, 96 score,
            agg = res.tile([H, NSHp], f32, name="agg")     # 97 ones
            dinv_bc = res.tile([H, NSHp], f32, name="dinv_bc")
            dinv_pp = res.tile([128, NT], f32, name="dinv_pp")
            iota16 = res.tile([128, 128], bf16, name="iota16")
            ident = res.tile([128, 128], f32, name="ident")
            idxg = res.tile([128, (slots[0] + slots[1]) // 16], i16,
                            name="idxg")
            dlg = res.tile([128, nblk[0] + nblk[1]], bf16, name="dlg")
            oh_sp = res.tile([128, K * 128], bf16, name="oh_sp")
            oht_sp = res.tile([128, K * 128], bf16, name="oht_sp")
            wproj = res.tile([cfg.IN, H], f32, name="wproj")
            w1 = res.tile([H, H], f32, name="w1")
            w2 = res.tile([H, H], f32, name="w2")
            wa1 = res.tile([H, AH], f32, name="wa1")
            wa2 = res.tile([AH, 1], f32, name="wa2")
            wm1 = res.tile([H, H], f32, name="wm1")
            wm2 = res.tile([H, OD], f32, name="wm2")
            pft = res.tile([H, 16], f32, name="pft")
            ab1 = res.tile([AH, 1], f32, name="ab1")
            ab2 = res.tile([1, 1], f32, name="ab2")
            mb2 = res.tile([OD, 1], f32, name="mb2")
            sq = res.tile([H, 512], f32, name="sq")
            vec = res.tile([H, 8], f32, name="vec")
            Rst = res.tile([128, NT * 98], bf16, name="Rst")  # spot rows
            Evec = res.tile([128, NT], bf16, name="Evec")     # exp(score)
            DNcol = res.tile([128, ST], bf16, name="DNcol")   # 1/denom cols
            DNf = res.tile([128, ST], f32, name="DNf")
            spot_fm = res.tile([H, ST * 128], f32, name="spot_fm")

            for t_, s_ in ((iota16, iota16_in), (ident, ident_in),
                           (idxg, idx_gcn), (dlg, dl_gcn),
                           (oh_sp, oh_sp_in), (oht_sp, oht_sp_in),
                           (wproj, projW), (w1, W1), (w2, W2),
                           (wa1, attnW1), (wa2, attnW2), (wm1, mlpW1),
                           (wm2, mlpW2), (pft, pf), (ab1, attn_b1),
                           (ab2, attn_b2), (mb2, mlp_b2)):
                nc.sync.dma_start(t_[:], s_[:])

            # dinv = rsqrt(deg); pad cols get deg=1e30 -> dinv ~ 0
            nc.sync.dma_start(dinv_pp[:], deg_pp[:])
            nc.scalar.activation(dinv_pp[:], dinv_pp[:],
                                 mybir.ActivationFunctionType.Sqrt)
            nc.vector.reciprocal(dinv_pp[:], dinv_pp[:])
            nc.sync.dma_start(dinv_bc[:], deg_bc[:])
            nc.scalar.activation(dinv_bc[:], dinv_bc[:],
                                 mybir.ActivationFunctionType.Sqrt)
            nc.vector.reciprocal(dinv_bc[:], dinv_bc[:])

            tbl_own = [dram.tile([NSHp, TROWB], bf16, name=f"tblo{i}")
                       for i in range(2)]
            tbl_full = [dram.tile([N_CORES * NSHp, TROWB], bf16,
                                  addr_space="Shared", name=f"tblf{i}")
                        for i in range(2)]
            st_in = [dram.tile([H, 2], f32, name=f"sti{i}") for i in range(4)]
            st_out = [dram.tile([H, 2], f32, addr_space="Shared",
                                name=f"sto{i}") for i in range(4)]

            NCHUNK = (NSHp + 511) // 512

            def csz(ci):
                return min(512, NSHp - ci * 512)

            def graph_norm_relu(dst_ap, u_ap, width, n_total, stats_idx,
                                pre_b_col, gn_cols):
                """dst = relu(S*u + B) with GN stats over u[:, :width]."""
                s1 = vec[:, 0:1]
                nc.vector.tensor_reduce(s1, u_ap[:, :width],
                                        mybir.AxisListType.X,
                                        mybir.AluOpType.add)
                nch = (width + 511) // 512
                s2p = res.tile([H, nch], f32, name=f"s2p{stats_idx}")
                for ci in range(nch):
                    w_ = min(512, width - ci * 512)
                    nc.scalar.activation(
                        sq[:, :w_], u_ap[:, ci * 512:ci * 512 + w_],
                        mybir.ActivationFunctionType.Square,
                        accum_out=s2p[:, ci:ci + 1])
                nc.vector.tensor_reduce(vec[:, 1:2], s2p[:],
                                        mybir.AxisListType.X,
                                        mybir.AluOpType.add)
                stv = stg.tile([H, 2], f32, name=f"stv{stats_idx}")
                nc.vector.tensor_copy(stv[:], vec[:, 0:2])
                nc.sync.dma_start(st_in[stats_idx][:], stv[:])
                nc.gpsimd.collective_compute(
                    "AllReduce", mybir.AluOpType.add,
                    replica_groups=[list(range(N_CORES))],
                    ins=[st_in[stats_idx][:].opt()],
                    outs=[st_out[stats_idx][:].opt()])
                stt = stg.tile([H, 2], f32, name=f"stt{stats_idx}")
                nc.sync.dma_start(stt[:], st_out[stats_idx][:])
                gw = pft[:, gn_cols[0]:gn_cols[0] + 1]
                gb = pft[:, gn_cols[1]:gn_cols[1] + 1]
                ga = pft[:, gn_cols[2]:gn_cols[2] + 1]
                mean = vec[:, 2:3]
                ex2 = vec[:, 3:4]
                inv_n = 1.0 / float(n_total)
                nc.vector.tensor_scalar(mean, stt[:, 0:1], inv_n, None,
                                        mybir.AluOpType.mult)
                nc.vector.tensor_scalar(ex2, stt[:, 1:2], inv_n, None,
                                        mybir.AluOpType.mult)
                if pre_b_col is not None:
                    c_ = pft[:, pre_b_col:pre_b_col + 1]
                    t0 = vec[:, 4:5]
                    nc.vector.tensor_tensor(t0, c_, mean, mybir.AluOpType.mult)
                    nc.vector.tensor_scalar(t0, t0, 2.0, None,
                                            mybir.AluOpType.mult)
                    nc.vector.tensor_tensor(ex2, ex2, t0, mybir.AluOpType.add)
                    t1 = vec[:, 5:6]
                    nc.vector.tensor_tensor(t1, c_, c_, mybir.AluOpType.mult)
                    nc.vector.tensor_tensor(ex2, ex2, t1, mybir.AluOpType.add)
                    nc.vector.tensor_tensor(mean, mean, c_, mybir.AluOpType.add)
                m2 = vec[:, 4:5]
                nc.vector.tensor_tensor(m2, mean, mean, mybir.AluOpType.mult)
                a2 = vec[:, 5:6]
                nc.vector.tensor_scalar(a2, ga, -1.0, 2.0,
                                        mybir.AluOpType.mult,
                                        mybir.AluOpType.add)
                nc.vector.tensor_tensor(a2, a2, ga, mybir.AluOpType.mult)
                nc.vector.tensor_tensor(m2, m2, a2, mybir.AluOpType.mult)
                var = vec[:, 6:7]
                nc.vector.tensor_tensor(var, ex2, m2,
                                        mybir.AluOpType.subtract)
                nc.vector.tensor_scalar(var, var, float(cfg.EPS), None,
                                        mybir.AluOpType.add)
                nc.scalar.activation(var, var,
                                     mybir.ActivationFunctionType.Sqrt)
                nc.vector.reciprocal(var, var)
                Sg = vec[:, 4:5]
                nc.vector.tensor_tensor(Sg, gw, var, mybir.AluOpType.mult)
                Bg = vec[:, 5:6]
                nc.vector.tensor_tensor(Bg, Sg, ga, mybir.AluOpType.mult)
                nc.vector.tensor_tensor(Bg, Bg, mean, mybir.AluOpType.mult)
                nc.vector.tensor_tensor(Bg, gb, Bg, mybir.AluOpType.subtract)
                if pre_b_col is not None:
                    c_ = pft[:, pre_b_col:pre_b_col + 1]
                    t0 = vec[:, 6:7]
                    nc.vector.tensor_tensor(t0, Sg, c_, mybir.AluOpType.mult)
                    nc.vector.tensor_tensor(Bg, Bg, t0, mybir.AluOpType.add)
                nc.scalar.activation(dst_ap, u_ap,
                                     mybir.ActivationFunctionType.Relu,
                                     bias=Bg, scale=Sg)

            # ================= proj layer =================
            nc.sync.dma_start(h[:cfg.IN, :], xT[:])
            for ci in range(NCHUNK):
                w_ = csz(ci)
                ps = mmp.tile([H, 512], f32, name=f"pj{ci}", tag="mm")
                nc.tensor.matmul(ps[:, :w_], wproj[:],
                                 h[:cfg.IN, ci * 512:ci * 512 + w_],
                                 start=True, stop=True)
                nc.vector.tensor_copy(agg[:, ci * 512:ci * 512 + w_],
                                      ps[:, :w_])
            graph_norm_relu(h[:H, :], agg[:], NSHp, cfg.N, 0, 0, (1, 2, 3))
            if DEBUG:
                nc.sync.dma_start(dbg_h0[:], h[:H, :])

            # ================= GCN layers =================
            for li, (Wt, b_col, gn_cols) in enumerate(
                    ((w1, 4, (5, 6, 7)), (w2, 8, (9, 10, 11)))):
                # table rows t' = dinv * (h @ W), bf16 [NSHp, 128]
                for t in range(NT):
                    ps = mmp.tile([128, H], f32, name=f"tb{li}_{t}", tag="mm")
                    nc.tensor.matmul(ps[:, :], h[:H, t * 128:(t + 1) * 128],
                                     Wt[:], start=True, stop=True)
                    sg = stg.tile([128, TROWB], bf16, name=f"ts{li}_{t}",
                                  tag="tstg")
                    nc.vector.memset(sg[:, H:], 0.0)
                    nc.vector.tensor_scalar(sg[:, :H], ps[:, :],
                                            dinv_pp[:, t:t + 1], None,
                                            mybir.AluOpType.mult)
                    nc.sync.dma_start(
                        tbl_own[li][t * 128:(t + 1) * 128, :], sg[:])
                nc.gpsimd.collective_compute(
                    "AllGather", mybir.AluOpType.bypass,
                    replica_groups=[list(range(N_CORES))],
                    ins=[tbl_own[li][:].opt()], outs=[tbl_full[li][:].opt()])
                # self-loop: agg = (W^T h) * dinv_bc
                for ci in range(NCHUNK):
                    w_ = csz(ci)
                    ps = mmp.tile([H, 512], f32, name=f"sf{li}_{ci}", tag="mm")
                    nc.tensor.matmul(ps[:, :w_], Wt[:],
                                     h[:H, ci * 512:ci * 512 + w_],
                                     start=True, stop=True)
                    nc.vector.tensor_tensor(
                        agg[:, ci * 512:ci * 512 + w_], ps[:, :w_],
                        dinv_bc[:, ci * 512:ci * 512 + w_],
                        mybir.AluOpType.mult)
                # gather + scatter, two phases (src halves).
                # dma_gather runs prepare_only: descriptor generation starts
                # during the AllGather (no table dependency); the trigger
                # carries the table read and fires as soon as it lands. The
                # first GBUF preps are emitted back to back so they fill the
                # collective window; after that each prep is paired with its
                # trigger (WAW on the gather tiles paces the pipeline).
                allcalls = []
                for hph in (0, 1):
                    tile_of = np.repeat(np.arange(NT), S[:, hph])
                    for k, (o, n) in enumerate(gcalls[hph]):
                        allcalls.append((hph, k, o, n, tile_of))
                qn = 1
                gtiles = {}
                st_cons = dict(open_ps=None, open_t=-1, prev_hph=0)

                def consume(ic):
                    hph, k, o, n, tile_of = allcalls[ic]
                    nb = n // 128
                    blk0 = 0 if hph == 0 else nblk[0]
                    if hph != st_cons['prev_hph']:
                        st_cons['open_t'] = -1
                        st_cons['prev_hph'] = hph
                    g = gtiles.pop(ic)
                    oh = ohp.tile([128, CB, 128], bf16,
                                  name=f"oh{li}_{hph}_{k}", tag="oh")
                    dlsl = dlg[:, blk0 + o // 128: blk0 + (o + n) // 128]
                    nc.vector.tensor_tensor(
                        oh[:, :nb, :],
                        iota16[:].unsqueeze(1).broadcast_to([128, nb, 128]),
                        dlsl.unsqueeze(2).broadcast_to([128, nb, 128]),
                        mybir.AluOpType.is_equal)
                    for j in range(nb):
                        b = o // 128 + j
                        t = int(tile_of[b])
                        if t != st_cons['open_t']:
                            st_cons['open_ps'] = scp.tile(
                                [H, 128], f32, name=f"sc{li}_{hph}_{b}",
                                tag="sc")
                            st_cons['open_t'] = t
                            first = True
                        else:
                            first = False
                        last = (b + 1 == len(tile_of)) or \
                               (tile_of[b + 1] != t)
                        nc.tensor.matmul(st_cons['open_ps'][:], g[:, j, :H],
                                         oh[:, j, :],
                                         start=first, stop=last)
                        if last:
                            nc.vector.tensor_tensor(
                                agg[:, t * 128:(t + 1) * 128],
                                agg[:, t * 128:(t + 1) * 128],
                                st_cons['open_ps'][:], mybir.AluOpType.add)

                for idx_call, (hph, k, o, n, tile_of) in enumerate(allcalls):
                    nb = n // 128
                    tview = tbl_full[li][hph * cfg.HALF:
                                         hph * cfg.HALF + cfg.HALF, :]
                    col0 = 0 if hph == 0 else slots[0] // 16
                    g = gat.tile([128, CB, TROWB], bf16,
                                 name=f"g{li}_{hph}_{k}", tag="gat")
                    gtiles[idx_call] = g
                    nc.gpsimd.dma_gather(
                        g[:, :nb, :], tview,
                        idxg[:, col0 + o // 16: col0 + (o + n) // 16],
                        n, n, TROWB, single_packet=False, queue_num=qn)
                    qn = 1 + (qn % 3)
                    consume(idx_call)
                # u = agg * dinv (per dst node)
                nc.vector.tensor_tensor(agg[:], agg[:], dinv_bc[:],
                                        mybir.AluOpType.mult)
                if DEBUG:
                    nc.sync.dma_start(dbg_ag[li][:], agg[:])
                graph_norm_relu(h[:H, :], agg[:], NSHp, cfg.N,
                                1 + li, b_col, gn_cols)
                if DEBUG:
                    nc.sync.dma_start((dbg_h1 if li == 0 else dbg_h2)[:],
                                      h[:H, :])

            # ================= attention scores =================
            # rows 96:98 <- 1.0 first; attn then overwrites row 96 with the
            # score, leaving row 97 as the ones row for the spot denominator
            # (a lone write at partition 97 breaks the 32-partition alignment
            # rule, so both rows are set together).
            nc.vector.memset(h[96:98, :], 1.0)
            for ci in range(NCHUNK):
                w_ = csz(ci)
                ps = mmp.tile([AH, 512], f32, name=f"at{ci}", tag="mm")
                nc.tensor.matmul(ps[:, :w_], wa1[:],
                                 h[:H, ci * 512:ci * 512 + w_],
                                 start=True, stop=True)
                uc = stg.tile([AH, 512], f32, name=f"uat{ci}", tag="uat")
                nc.scalar.activation(uc[:, :w_], ps[:, :w_],
                                     mybir.ActivationFunctionType.Relu,
                                     bias=ab1[:])
                ps2 = mmp.tile([1, 512], f32, name=f"sc2{ci}", tag="mm1")
                nc.tensor.matmul(ps2[:, :w_], wa2[:], uc[:, :w_],
                                 start=True, stop=True)
                nc.vector.tensor_scalar(h[H:H + 1, ci * 512:ci * 512 + w_],
                                        ps2[:, :w_], ab2[:],
                                        None, mybir.AluOpType.add)
            if DEBUG:
                nc.sync.dma_start(dbg_sc[:], h[H:H + 1, :])

            # ================= spot aggregation (core-local) ==============
            # R_g = transpose(h[0:98, tile]) node rows [h | score | 1];
            # e = exp(score); den[s] = OH^T e (column); dn = OHT^T (1/den);
            # spot_fm[:, sg] = sum_g (R_g * e * dn)^T OH_{g,sg}
            for g_ in range(NT):
                pt = mmp.tile([128, 98], f32, name=f"tr{g_}", tag="tpose")
                nc.tensor.transpose(pt[:, :98], h[:98, g_ * 128:(g_ + 1) * 128],
                                    ident[:98, :98])
                nc.vector.tensor_copy(Rst[:, g_ * 98:(g_ + 1) * 98], pt[:])
                nc.scalar.activation(Evec[:, g_:g_ + 1], pt[:, 96:97],
                                     mybir.ActivationFunctionType.Exp)
            # denominators per spot group: den[s] = sum_n e_n OH[n, s]
            for sg in range(ST):
                ks = [k for k, (g_, s_) in enumerate(spairs) if s_ == sg]
                dps = mmp.tile([128, 1], f32, name=f"dn{sg}", tag="mm1")
                for i, k in enumerate(ks):
                    g_ = spairs[k][0]
                    nc.tensor.matmul(dps[:], oh_sp[:, k * 128:(k + 1) * 128],
                                     Evec[:, g_:g_ + 1],
                                     start=(i == 0), stop=(i == len(ks) - 1))
                nc.vector.tensor_scalar(DNf[:, sg:sg + 1], dps[:], 1e-30,
                                        None, mybir.AluOpType.max)
            nc.vector.reciprocal(DNf[:], DNf[:])
            nc.vector.tensor_copy(DNcol[:], DNf[:])
            if DEBUG:
                nc.sync.dma_start(dbg_den[:], DNf[:])
            # weighted rows and numerators
            wcol = res.tile([128, NT], f32, name="wcol")
            for g_ in range(NT):
                ks = [k for k, (gg, s_) in enumerate(spairs) if gg == g_]
                dn = mmp.tile([128, 1], f32, name=f"dnn{g_}", tag="mm1")
                for i, k in enumerate(ks):
                    sg = spairs[k][1]
                    nc.tensor.matmul(dn[:], oht_sp[:, k * 128:(k + 1) * 128],
                                     DNcol[:, sg:sg + 1],
                                     start=(i == 0), stop=(i == len(ks) - 1))
                nc.vector.tensor_tensor(wcol[:, g_:g_ + 1],
                                        Evec[:, g_:g_ + 1], dn[:],
                                        mybir.AluOpType.mult)
            Rw = res.tile([128, NT * 98], bf16, name="Rw")
            for g_ in range(NT):
                nc.vector.tensor_scalar(Rw[:, g_ * 98:(g_ + 1) * 98],
                                        Rst[:, g_ * 98:(g_ + 1) * 98],
                                        wcol[:, g_:g_ + 1], None,
                                        mybir.AluOpType.mult)
            for sg in range(ST):
                ks = [k for k, (g_, s_) in enumerate(spairs) if s_ == sg]
                sps = mmp.tile([98, 128], f32, name=f"sp{sg}", tag="tpose")
                for i, k in enumerate(ks):
                    g_ = spairs[k][0]
                    nc.tensor.matmul(sps[:], Rw[:, g_ * 98:(g_ + 1) * 98],
                                     oh_sp[:, k * 128:(k + 1) * 128],
                                     start=(i == 0), stop=(i == len(ks) - 1))
                nc.vector.tensor_copy(spot_fm[:, sg * 128:(sg + 1) * 128],
                                      sps[:H, :])
            if DEBUG:
                nc.sync.dma_start(dbg_sp[:], spot_fm[:])

            # ================= MLP head =================
            um = res.tile([H, ST * 128], f32, name="um")
            for ci in range((ST * 128 + 511) // 512):
                w_ = min(512, ST * 128 - ci * 512)
                ps = mmp.tile([H, 512], f32, name=f"m1{ci}", tag="mm")
                nc.tensor.matmul(ps[:, :w_], wm1[:],
                                 spot_fm[:, ci * 512:ci * 512 + w_],
                                 start=True, stop=True)
                nc.vector.tensor_copy(um[:, ci * 512:ci * 512 + w_],
                                      ps[:, :w_])
            graph_norm_relu(um[:], um[:], SSH, cfg.S, 3, 12, (13, 14, 15))
            zo = res.tile([OD, ST * 128], f32, name="zo")
            for ci in range((ST * 128 + 511) // 512):
                w_ = min(512, ST * 128 - ci * 512)
                ps = mmp.tile([OD, 512], f32, name=f"m2{ci}", tag="mm")
                nc.tensor.matmul(ps[:, :w_], wm2[:],
                                 um[:, ci * 512:ci * 512 + w_],
                                 start=True, stop=True)
                nc.vector.tensor_scalar(zo[:, ci * 512:ci * 512 + w_],
                                        ps[:, :w_], mb2[:], None,
                                        mybir.AluOpType.add)
            for gi in range(ST):
                n_ = min(128, SSH - gi * 128)
                if n_ <= 0:
                    break
                ps = mmp.tile([128, OD], f32, name=f"ot{gi}", tag="tpose")
                nc.tensor.transpose(ps[:, :], zo[:, gi * 128:(gi + 1) * 128],
                                    ident[:OD, :OD])
                sg = stg.tile([128, OD], f32, name=f"os{gi}", tag="ostg")
                nc.vector.tensor_copy(sg[:], ps[:])
                nc.sync.dma_start(out[gi * 128:gi * 128 + n_, :], sg[:n_, :])

    nc.compile()
    return nc


_CACHE = {}


def _build_inputs(cfg, inputs, perm, starts, counts, idx_w, dl_w,
                  oh_l, oht_l, deg_new):
    f = np.float32
    import ml_dtypes
    x = np.asarray(inputs['x'], f)
    NSHp, NT, H = cfg.NSHp, cfg.NT, cfg.H

    def col(v):
        return np.asarray(v, f).reshape(-1, 1)

    pf = np.zeros((H, 16), f)
    for i, k in enumerate(['proj_b', 'gn0_w', 'gn0_b', 'gn0_a',
                           'gcn1_b', 'gn1_w', 'gn1_b', 'gn1_a',
                           'gcn2_b', 'gn2_w', 'gn2_b', 'gn2_a',
                           'mlp_b1', 'mlp_gn_w', 'mlp_gn_b', 'mlp_gn_a']):
        pf[:, i] = np.asarray(inputs[k], f)
    iota = np.broadcast_to(np.arange(128, dtype=f), (128, 128)).copy()
    iota16 = iota.astype(ml_dtypes.bfloat16)
    ident = np.eye(128, dtype=f)
    in_maps = []
    for c in range(N_CORES):
        cnt = int(counts[c])
        own = perm[starts[c]:starts[c] + cnt]       # old ids, new order
        xc = np.zeros((cfg.IN, NSHp), f)
        xc[:, :cnt] = x[own].T
        deg_own = np.full(NSHp, 1e30, f)
        deg_own[:cnt] = deg_new[starts[c]:starts[c] + cnt]
        dpp = np.full((128, NT), 1e30, f)
        for t in range(NT):
            dpp[:, t] = deg_own[t * 128:(t + 1) * 128]
        dbc = np.broadcast_to(deg_own[None, :], (H, NSHp)).copy()
        in_maps.append({
            'xT': xc, 'deg_pp': dpp, 'deg_bc': dbc,
            'iota16_in': iota16, 'ident_in': ident,
            'idx_gcn': idx_w[c],
            'dl_gcn': dl_w[c].astype(ml_dtypes.bfloat16),
            'oh_sp_in': oh_l[c].astype(ml_dtypes.bfloat16),
            'oht_sp_in': oht_l[c].astype(ml_dtypes.bfloat16),
            'projW': np.asarray(inputs['proj_W'], f),
            'W1': np.asarray(inputs['gcn1_W'], f),
            'W2': np.asarray(inputs['gcn2_W'], f),
            'attnW1': np.asarray(inputs['attn_W1'], f),
            'attnW2': np.asarray(inputs['attn_W2'], f),
            'mlpW1': np.asarray(inputs['mlp_W1'], f),
            'mlpW2': np.asarray(inputs['mlp_W2'], f),
            'pf': pf,
            'attn_b1': col(inputs['attn_b1']),
            'attn_b2': col(inputs['attn_b2']),
            'mlp_b2': col(inputs['mlp_b2']),
        })
    return in_maps


def kernel(**inputs):
    from concourse import bass_utils
    cfg = Cfg(n_nodes=int(np.asarray(inputs['x']).shape[0]),
              n_edges=int(np.asarray(inputs['edge_index']).shape[1]),
              in_dim=int(np.asarray(inputs['x']).shape[1]),
              hid=int(np.asarray(inputs['proj_W']).shape[1]),
              attn_hid=int(np.asarray(inputs['attn_W1']).shape[1]),
              out_dim=int(np.asarray(inputs['mlp_W2']).shape[1]),
              n_spots=int(inputs['num_spots']))
    ei = np.asarray(inputs['edge_index']).astype(np.int64)
    cts = np.asarray(inputs['cell_to_spot']).astype(np.int64)
    src, dst = ei[0], ei[1]

    perm, starts, counts, spot_sorted = _relabel(cfg, cts)
    inv = np.empty(cfg.N, np.int64)
    inv[perm] = np.arange(cfg.N)
    src_new, dst_new = inv[src], inv[dst]
    # node -> core and global table row id
    node_core = np.searchsorted(starts, np.arange(cfg.N), side='right') - 1
    # starts from _relabel are in new-id space: node i (new) on core c iff
    # starts[c] <= i < starts[c]+counts[c]
    loc = np.arange(cfg.N) - starts[node_core]
    gid = node_core * cfg.NSHp + loc
    src_gid = gid[src_new]
    deg_full = (np.bincount(dst_new, minlength=cfg.N) + 1).astype(np.float32)

    idx_w, dl_w, emeta = _prep_edges(cfg, src_gid, dst_new, node_core, starts)
    spairs, oh_l, oht_l = _prep_spot_oh(cfg, spot_sorted, starts, counts)

    key = (cfg.N, cfg.E, cfg.NSHp, tuple(emeta['nblk']), len(spairs))
    if key not in _CACHE:
        _CACHE[key] = build_program(cfg, emeta, spairs)
    nc = _CACHE[key]

    in_maps = _build_inputs(cfg, inputs, perm, starts, counts, idx_w, dl_w,
                            oh_l, oht_l, deg_full)
    res = bass_utils.run_bass_kernel_spmd(
        nc, in_maps, core_ids=list(range(N_CORES)),
        trace=os.environ.get('KERNEL_TRACE', '0') == '1',
        tmpdir=os.environ.get('KERNEL_TMPD'))
    if os.environ.get('KERNEL_TRACE', '0') == '1':
        print('HW exec time:', res.exec_time_ns, 'ns')
    out = np.concatenate([res.results[c]['out'] for c in range(N_CORES)],
                         axis=0)
    return out.astype(np.float32)
